# revision 10
# baseline (speedup 1.0000x reference)
"""Trainium2 Bass kernel for soft-argmax ROI bilinear sampling.

Problem (hardcoded):
  images:      (64, 3, 224, 224) f32
  feature_map: (64, 16, 56, 56) f32
  outputs: roi (64, 3, 16, 128, 128), mean_x/mean_y/scale (64, 16), all f32

Math: per (b, c): softmax over the 56x56 feature map gives soft-argmax
mean_x/mean_y and an L1 spread 'scale'.  A 128x128 sampling grid is
bilinear-sampled from images[b].  The sample coordinates are separable
(x coords depend only on output col j, y coords only on output row i), so
bilinear sampling == two small dense matmuls with "hat" weight matrices:
    roi[b,ci,c] = Wy[b,c] @ images[b,ci] @ Wx[b,c]^T
where W[o, s] = relu(1 - |r_o - s|) for in-range coords, 0 otherwise
(matching the reference's clipped ceil/floor behavior, including coords
exactly on integers producing 0).  We build W negated (min(|t|-1, 0)) so a
single fused tensor_scalar produces it; the two negations cancel.

Sharding: pure data parallel over batch, 8 batches per core x 8 cores.
"""

import os
from contextlib import ExitStack

import numpy as np

import concourse.bass as bass
import concourse.bacc as bacc
import concourse.tile as tile
import concourse.mybir as mybir
from concourse.bass_utils import run_bass_kernel_spmd

F32 = mybir.dt.float32
ALU = mybir.AluOpType

# ---- problem dims (hardcoded per contract) ----
B, CI, YI, XI = 64, 3, 224, 224
C, YF, XF = 16, 56, 56
OY = OX = 128
NCORES = 8
BL = B // NCORES          # 8 local batches
NP = BL * C               # 128 = (b,c) pairs per core -> partition dim
FDIM = YF * XF            # 3136

# K tiling of the 224-long source axes: 128 + 96
K0, K1 = 128, 96
NCI = 2048                # (c, i) packed free size = C * OY

# matmul dtype for the heavy stages (images / W / intermediate).
# float32 = exact; bfloat16 = ~2x faster PE, ~0.5% rel err.
MM_DT = mybir.dt.float32
OOB = 1.0e4

_CACHE = {}

LAST_EXEC_NS = None
LAST_RESULTS = None


def _build_program():
    nc = bacc.Bacc("TRN2", target_bir_lowering=False, debug=False,
                   enable_asserts=False, num_devices=NCORES)

    # ---- DRAM I/O ----
    images_d = nc.dram_tensor("images", [BL, CI, YI, XI], F32, kind="ExternalInput").ap()
    fm_d = nc.dram_tensor("feature_map", [BL, C, YF, XF], F32, kind="ExternalInput").ap()
    c_gx = nc.dram_tensor("c_gx", [XF], F32, kind="ExternalInput")
    c_gy = nc.dram_tensor("c_gy", [YF], F32, kind="ExternalInput")
    c_grid = nc.dram_tensor("c_grid", [OY], F32, kind="ExternalInput")
    c_niota = nc.dram_tensor("c_niota", [YI], F32, kind="ExternalInput")

    roi_d = nc.dram_tensor("roi", [BL, CI, C, OY, OX], F32, kind="ExternalOutput").ap()
    meanx_d = nc.dram_tensor("mean_x", [BL, C], F32, kind="ExternalOutput").ap()
    meany_d = nc.dram_tensor("mean_y", [BL, C], F32, kind="ExternalOutput").ap()
    scale_d = nc.dram_tensor("scale", [BL, C], F32, kind="ExternalOutput").ap()

    def bc(handle, n, p=128):
        # DRAM vector broadcast across p partitions: AP [[0,p],[1,n]]
        return bass.AP(tensor=handle, offset=0, ap=[[0, p], [1, n]])

    with tile.TileContext(nc) as tc, ExitStack() as ctx:
        consts = ctx.enter_context(tc.tile_pool(name="consts", bufs=1))
        fmp = ctx.enter_context(tc.tile_pool(name="fmp", bufs=1))
        stats = ctx.enter_context(tc.tile_pool(name="stats", bufs=1))
        imgp = ctx.enter_context(tc.tile_pool(name="imgp", bufs=2))
        flats = ctx.enter_context(tc.tile_pool(name="flats", bufs=2))
        reps = ctx.enter_context(tc.tile_pool(name="reps", bufs=2))
        wabs = ctx.enter_context(tc.tile_pool(name="wabs", bufs=2))
        wp = ctx.enter_context(tc.tile_pool(name="wp", bufs=1))
        o1p = ctx.enter_context(tc.tile_pool(name="o1p", bufs=1))
        rcp = ctx.enter_context(tc.tile_pool(name="rcp", bufs=4))
        ps1 = ctx.enter_context(tc.tile_pool(name="ps1", bufs=3, space="PSUM"))
        ps2 = ctx.enter_context(tc.tile_pool(name="ps2", bufs=2, space="PSUM"))

        # ---- constants to SBUF ----
        gxc = consts.tile([NP, XF], F32)
        nc.sync.dma_start(out=gxc, in_=bc(c_gx, XF))
        gyc = consts.tile([NP, YF], F32)
        nc.sync.dma_start(out=gyc, in_=bc(c_gy, YF))
        gridc = consts.tile([NP, OY], F32)
        nc.sync.dma_start(out=gridc, in_=bc(c_grid, OY))
        nio0 = consts.tile([K0, 1], F32)
        nc.sync.dma_start(out=nio0, in_=c_niota.ap()[0:K0].unsqueeze(-1))
        nio1 = consts.tile([K1, 1], F32)
        nc.sync.dma_start(out=nio1, in_=c_niota.ap()[K0:YI].unsqueeze(-1))

        # ---- feature map as (128, 3136); softmax done in-place ----
        FM = fmp.tile([NP, FDIM], F32)
        nc.sync.dma_start(out=FM, in_=fm_d.rearrange("b c y x -> (b c) (y x)"))

        mx = stats.tile([NP, 1], F32)
        nc.vector.reduce_max(mx, FM, axis=mybir.AxisListType.X)
        nmx = stats.tile([NP, 1], F32)
        nc.vector.tensor_scalar_mul(nmx, mx, -1.0)
        Z = stats.tile([NP, 1], F32)
        nc.scalar.activation(FM, FM, mybir.ActivationFunctionType.Exp,
                             bias=nmx, scale=1.0, accum_out=Z)
        Zr = stats.tile([NP, 1], F32)
        nc.vector.reciprocal(Zr, Z)
        # full normalization (keeps boundary decisions aligned w/ reference)
        nc.vector.tensor_scalar(FM, FM, Zr, None, op0=ALU.mult)

        SMyx = FM.rearrange("p (y x) -> p y x", y=YF)
        rowsum = stats.tile([NP, YF], F32)
        nc.vector.reduce_sum(rowsum, SMyx, axis=mybir.AxisListType.X)
        colsum = stats.tile([NP, XF], F32)
        nc.vector.reduce_sum(colsum, SMyx.rearrange("p y x -> p x y"),
                             axis=mybir.AxisListType.X)

        mxw = stats.tile([NP, XF], F32)
        nc.vector.tensor_tensor(mxw, colsum, gxc, op=ALU.mult)
        mean_x = stats.tile([NP, 1], F32)
        nc.vector.reduce_sum(mean_x, mxw, axis=mybir.AxisListType.X)
        myw = stats.tile([NP, YF], F32)
        nc.vector.tensor_tensor(myw, rowsum, gyc, op=ALU.mult)
        mean_y = stats.tile([NP, 1], F32)
        nc.vector.reduce_sum(mean_y, myw, axis=mybir.AxisListType.X)

        dxt = stats.tile([NP, XF], F32)
        nc.vector.tensor_scalar(dxt, gxc, mean_x, None, op0=ALU.subtract)
        dxw = stats.tile([NP, XF], F32)
        nc.vector.tensor_tensor(dxw, dxt, colsum, op=ALU.mult)
        sx = stats.tile([NP, 1], F32)
        nc.vector.reduce_sum(sx, dxw, axis=mybir.AxisListType.X,
                             apply_absolute_value=True)
        dyt = stats.tile([NP, YF], F32)
        nc.vector.tensor_scalar(dyt, gyc, mean_y, None, op0=ALU.subtract)
        dyw = stats.tile([NP, YF], F32)
        nc.vector.tensor_tensor(dyw, dyt, rowsum, op=ALU.mult)
        sy = stats.tile([NP, 1], F32)
        nc.vector.reduce_sum(sy, dyw, axis=mybir.AxisListType.X,
                             apply_absolute_value=True)
        scale_t = stats.tile([NP, 1], F32)
        nc.vector.tensor_tensor(scale_t, sx, sy, op=ALU.add)

        # stat outputs
        nc.sync.dma_start(out=meanx_d.rearrange("b c -> (b c)").unsqueeze(-1), in_=mean_x)
        nc.sync.dma_start(out=meany_d.rearrange("b c -> (b c)").unsqueeze(-1), in_=mean_y)
        nc.sync.dma_start(out=scale_d.rearrange("b c -> (b c)").unsqueeze(-1), in_=scale_t)

        # ---- sampling coords (pixel space), masked ----
        def coords(mean_t, name):
            ind = stats.tile([NP, OY], F32, name=f"ind_{name}")
            # ind = grid*scale + mean   (same op order as reference)
            nc.vector.tensor_scalar(ind, gridc, scale_t, mean_t,
                                    op0=ALU.mult, op1=ALU.add)
            r = stats.tile([NP, OY], F32, name=f"r_{name}")
            # r = (ind + 1) * 112
            nc.vector.tensor_scalar(r, ind, 1.0, YI / 2.0,
                                    op0=ALU.add, op1=ALU.mult)
            m1 = stats.tile([NP, OY], F32, name=f"m1_{name}")
            nc.vector.tensor_scalar(m1, r, 0.0, None, op0=ALU.is_ge)
            m2 = stats.tile([NP, OY], F32, name=f"m2_{name}")
            nc.vector.tensor_scalar(m2, r, float(YI - 1), None, op0=ALU.is_le)
            # exact-integer coords produce 0 in the reference (ceil==floor):
            # round(r) via the 2^23 trick, then ne = 1 - (r == round(r))
            rnd = stats.tile([NP, OY], F32, name=f"rnd_{name}")
            nc.vector.tensor_scalar(rnd, r, 8388608.0, -8388608.0,
                                    op0=ALU.add, op1=ALU.add)
            eq = stats.tile([NP, OY], F32, name=f"eq_{name}")
            nc.vector.tensor_tensor(eq, r, rnd, op=ALU.is_equal)
            ne = stats.tile([NP, OY], F32, name=f"ne_{name}")
            nc.vector.tensor_scalar(ne, eq, -1.0, 1.0, op0=ALU.mult, op1=ALU.add)
            m = stats.tile([NP, OY], F32, name=f"m_{name}")
            nc.vector.tensor_tensor(m, m1, m2, op=ALU.mult)
            nc.vector.tensor_tensor(m, m, ne, op=ALU.mult)
            big = stats.tile([NP, OY], F32, name=f"big_{name}")
            nc.vector.tensor_scalar(big, m, -OOB, OOB, op0=ALU.mult, op1=ALU.add)
            rm = stats.tile([NP, OY], F32, name=f"rm_{name}")
            nc.vector.tensor_tensor(rm, r, m, op=ALU.mult)
            nc.vector.tensor_tensor(rm, rm, big, op=ALU.add)
            return rm

        ym = coords(mean_y, "y")
        xm = coords(mean_x, "x")

        KS = [(K0, 0, nio0), (K1, K0, nio1)]

        # ---- per-batch: image load, W build, stage 1, stage 2 ----
        for b in range(BL):
            # flatten masked coords to a single partition-0 row (1, 2048)
            # (partition_broadcast requires base partition 0)
            fy = flats.tile([1, NCI], F32, tag="fy")
            nc.sync.dma_start(
                out=fy.rearrange("p (c i) -> p c i", c=C),
                in_=ym[b * C:(b + 1) * C, :])
            fx = flats.tile([1, NCI], F32, tag="fx")
            nc.sync.dma_start(
                out=fx.rearrange("p (c i) -> p c i", c=C),
                in_=xm[b * C:(b + 1) * C, :])
            # image slab for this batch (src rows split 128 + 96)
            img0 = imgp.tile([K0, CI, XI], MM_DT, tag="img0")
            img1 = imgp.tile([K1, CI, XI], MM_DT, tag="img1")
            ieng = nc.sync if MM_DT == F32 else nc.gpsimd
            for ci in range(CI):
                ieng.dma_start(out=img0[:, ci, :], in_=images_d[b, ci, 0:K0, :])
                ieng.dma_start(out=img1[:, ci, :], in_=images_d[b, ci, K0:YI, :])

            # broadcast coord rows to all partitions
            yrep = reps.tile([K0, NCI], F32, tag="yrep")
            nc.gpsimd.partition_broadcast(yrep, fy)
            xrep = reps.tile([K0, NCI], F32, tag="xrep")
            nc.gpsimd.partition_broadcast(xrep, fx)

            wy = []
            wx = []
            for k, (kp, koff, nio) in enumerate(KS):
                a = wabs.tile([K0, NCI], F32, tag="wabs")
                # |r - s| in one ACT pass: Abs(yrep + (-iota))
                nc.scalar.activation(a[0:kp, :], yrep[0:kp, :],
                                     mybir.ActivationFunctionType.Abs,
                                     bias=nio, scale=1.0)
                w = wp.tile([kp, NCI], MM_DT, tag=f"wy{k}")
                # -hat = min(|t| - 1, 0)
                nc.vector.tensor_scalar(w, a[0:kp, :], 1.0, 0.0,
                                        op0=ALU.subtract, op1=ALU.min)
                wy.append(w)

                a2 = wabs.tile([K0, NCI], F32, tag="wabs")
                nc.scalar.activation(a2[0:kp, :], xrep[0:kp, :],
                                     mybir.ActivationFunctionType.Abs,
                                     bias=nio, scale=1.0)
                w2 = wp.tile([kp, NCI], MM_DT, tag=f"wx{k}")
                nc.vector.tensor_scalar(w2, a2[0:kp, :], 1.0, 0.0,
                                        op0=ALU.subtract, op1=ALU.min)
                wx.append(w2)

            # stage 1: o1[x, (c,i)] = sum_y img[y, x] * WyT[y, (c,i)]
            o1 = []
            for ci in range(CI):
                o1a = o1p.tile([K0, NCI], MM_DT, tag=f"o1a{ci}")
                o1b = o1p.tile([K1, NCI], MM_DT, tag=f"o1b{ci}")
                for m, (mp, moff, osb) in enumerate([(K0, 0, o1a), (K1, K0, o1b)]):
                    for h in range(2):          # halves of 2048
                        pA = ps1.tile([128, 1024], F32, tag="pA")
                        for q in range(2):      # 512-wide matmuls
                            nlo = h * 1024 + q * 512
                            for k, (kp, koff, io) in enumerate(KS):
                                nc.tensor.matmul(
                                    pA[0:mp, q * 512:(q + 1) * 512],
                                    (img0 if k == 0 else img1)[:, ci, moff:moff + mp],
                                    wy[k][:, nlo:nlo + 512],
                                    start=(k == 0), stop=(k == 1),
                                )
                        # drain (cast to MM_DT)
                        dst = osb[:, h * 1024:(h + 1) * 1024]
                        if (m + h) % 2 == 0:
                            nc.scalar.copy(dst, pA[0:mp, :])
                        else:
                            nc.vector.tensor_copy(dst, pA[0:mp, :])
                o1.append((o1a, o1b))

            # stage 2: roi[b,ci,c,i,j] = sum_x o1[x,(c,i)] * WxT[x,(c,j)]
            for c in range(C):
                pB = ps2.tile([128, CI, OX], F32, tag="pB")
                for ci in range(CI):
                    o1a, o1b = o1[ci]
                    for k in range(2):
                        lhs = (o1a if k == 0 else o1b)[:, c * OY:(c + 1) * OY]
                        rhs = wx[k][:, c * OX:(c + 1) * OX]
                        nc.tensor.matmul(
                            pB[:, ci, :], lhs, rhs,
                            start=(k == 0), stop=(k == 1),
                        )
                rc = rcp.tile([128, CI, OX], F32, tag="rc")
                if c % 2 == 0:
                    nc.scalar.copy(rc, pB)
                else:
                    nc.vector.tensor_copy(rc, pB)
                nc.sync.dma_start(
                    out=roi_d[b, :, c, :, :].rearrange("ci i j -> i ci j"),
                    in_=rc)

    nc.compile()
    return nc


def _get_nc():
    if "nc" not in _CACHE:
        _CACHE["nc"] = _build_program()
    return _CACHE["nc"]


def _consts_np():
    return {
        "c_gx": np.linspace(-1, 1, XF, dtype=np.float64).astype(np.float32),
        "c_gy": np.linspace(-1, 1, YF, dtype=np.float64).astype(np.float32),
        "c_grid": np.linspace(-1, 1, OY, dtype=np.float64).astype(np.float32),
        "c_niota": -np.arange(YI, dtype=np.float32),
    }


def kernel(images, feature_map):
    global LAST_EXEC_NS, LAST_RESULTS
    images = np.ascontiguousarray(np.asarray(images, dtype=np.float32))
    feature_map = np.ascontiguousarray(np.asarray(feature_map, dtype=np.float32))
    assert images.shape == (B, CI, YI, XI)
    assert feature_map.shape == (B, C, YF, XF)

    nc = _get_nc()
    consts = _consts_np()
    in_maps = []
    for s in range(NCORES):
        lo, hi = s * BL, (s + 1) * BL
        m = {"images": np.ascontiguousarray(images[lo:hi]),
             "feature_map": np.ascontiguousarray(feature_map[lo:hi])}
        m.update(consts)
        in_maps.append(m)

    trace = bool(int(os.environ.get("KERNEL_TRACE", "0")))
    res = run_bass_kernel_spmd(nc, in_maps, core_ids=list(range(NCORES)),
                               trace=trace)
    LAST_EXEC_NS = res.exec_time_ns
    LAST_RESULTS = res

    roi = np.concatenate([res.results[i]["roi"] for i in range(NCORES)], axis=0)
    mean_x = np.concatenate([res.results[i]["mean_x"] for i in range(NCORES)], axis=0)
    mean_y = np.concatenate([res.results[i]["mean_y"] for i in range(NCORES)], axis=0)
    scale = np.concatenate([res.results[i]["scale"] for i in range(NCORES)], axis=0)
    return roi, mean_x, mean_y, scale


# revision 11
# speedup vs baseline: 1.7873x; 1.7873x over previous
"""Trainium2 Bass kernel for soft-argmax ROI bilinear sampling.

Problem (hardcoded):
  images:      (64, 3, 224, 224) f32
  feature_map: (64, 16, 56, 56) f32
  outputs: roi (64, 3, 16, 128, 128), mean_x/mean_y/scale (64, 16), all f32

Math: per (b, c): softmax over the 56x56 feature map gives soft-argmax
mean_x/mean_y and an L1 spread 'scale'.  A 128x128 sampling grid is
bilinear-sampled from images[b].  The sample coordinates are separable
(x coords depend only on output col j, y coords only on output row i), so
bilinear sampling == two small dense matmuls with "hat" weight matrices:
    roi[b,ci,c] = Wy[b,c] @ images[b,ci] @ Wx[b,c]^T
where W[o, s] = relu(1 - |r_o - s|) for in-range coords, 0 otherwise
(matching the reference's clipped ceil/floor behavior, including coords
exactly on integers producing 0).  We build W negated (min(|t|-1, 0)) so a
single fused tensor_scalar produces it; the two negations cancel.

Sharding: pure data parallel over batch, 8 batches per core x 8 cores.
"""

import os
from contextlib import ExitStack

import numpy as np

import concourse.bass as bass
import concourse.bacc as bacc
import concourse.tile as tile
import concourse.mybir as mybir
from concourse.bass_utils import run_bass_kernel_spmd

F32 = mybir.dt.float32
ALU = mybir.AluOpType

# ---- problem dims (hardcoded per contract) ----
B, CI, YI, XI = 64, 3, 224, 224
C, YF, XF = 16, 56, 56
OY = OX = 128
NCORES = 8
BL = B // NCORES          # 8 local batches
NP = BL * C               # 128 = (b,c) pairs per core -> partition dim
FDIM = YF * XF            # 3136

# K tiling of the 224-long source axes: 128 + 96
K0, K1 = 128, 96
NCI = 2048                # (c, i) packed free size = C * OY

# matmul dtype for the heavy stages (images / W / intermediate).
# float32 = exact; bfloat16 = ~2x faster PE, ~0.5% rel err.
MM_DT = mybir.dt.bfloat16
OOB = 1.0e4

_CACHE = {}

LAST_EXEC_NS = None
LAST_RESULTS = None


def _build_program():
    nc = bacc.Bacc("TRN2", target_bir_lowering=False, debug=False,
                   enable_asserts=False, num_devices=NCORES)

    # ---- DRAM I/O ----
    images_d = nc.dram_tensor("images", [BL, CI, YI, XI], F32, kind="ExternalInput").ap()
    fm_d = nc.dram_tensor("feature_map", [BL, C, YF, XF], F32, kind="ExternalInput").ap()
    c_gx = nc.dram_tensor("c_gx", [XF], F32, kind="ExternalInput")
    c_gy = nc.dram_tensor("c_gy", [YF], F32, kind="ExternalInput")
    c_grid = nc.dram_tensor("c_grid", [OY], F32, kind="ExternalInput")
    c_niota = nc.dram_tensor("c_niota", [YI], F32, kind="ExternalInput")

    roi_d = nc.dram_tensor("roi", [BL, CI, C, OY, OX], F32, kind="ExternalOutput").ap()
    meanx_d = nc.dram_tensor("mean_x", [BL, C], F32, kind="ExternalOutput").ap()
    meany_d = nc.dram_tensor("mean_y", [BL, C], F32, kind="ExternalOutput").ap()
    scale_d = nc.dram_tensor("scale", [BL, C], F32, kind="ExternalOutput").ap()

    def bc(handle, n, p=128):
        # DRAM vector broadcast across p partitions: AP [[0,p],[1,n]]
        return bass.AP(tensor=handle, offset=0, ap=[[0, p], [1, n]])

    with tile.TileContext(nc) as tc, ExitStack() as ctx:
        consts = ctx.enter_context(tc.tile_pool(name="consts", bufs=1))
        fmp = ctx.enter_context(tc.tile_pool(name="fmp", bufs=1))
        stats = ctx.enter_context(tc.tile_pool(name="stats", bufs=1))
        imgp = ctx.enter_context(tc.tile_pool(name="imgp", bufs=2))
        flats = ctx.enter_context(tc.tile_pool(name="flats", bufs=2))
        reps = ctx.enter_context(tc.tile_pool(name="reps", bufs=2))
        wabs = ctx.enter_context(tc.tile_pool(name="wabs", bufs=2))
        wp = ctx.enter_context(tc.tile_pool(name="wp", bufs=1))
        o1p = ctx.enter_context(tc.tile_pool(name="o1p", bufs=1))
        rcp = ctx.enter_context(tc.tile_pool(name="rcp", bufs=4))
        ps1 = ctx.enter_context(tc.tile_pool(name="ps1", bufs=3, space="PSUM"))
        ps2 = ctx.enter_context(tc.tile_pool(name="ps2", bufs=2, space="PSUM"))

        # ---- constants to SBUF ----
        gxc = consts.tile([NP, XF], F32)
        nc.sync.dma_start(out=gxc, in_=bc(c_gx, XF))
        gyc = consts.tile([NP, YF], F32)
        nc.sync.dma_start(out=gyc, in_=bc(c_gy, YF))
        gridc = consts.tile([NP, OY], F32)
        nc.sync.dma_start(out=gridc, in_=bc(c_grid, OY))
        nio0 = consts.tile([K0, 1], F32)
        nc.sync.dma_start(out=nio0, in_=c_niota.ap()[0:K0].unsqueeze(-1))
        nio1 = consts.tile([K1, 1], F32)
        nc.sync.dma_start(out=nio1, in_=c_niota.ap()[K0:YI].unsqueeze(-1))

        # ---- feature map as (128, 3136); softmax done in-place ----
        FM = fmp.tile([NP, FDIM], F32)
        nc.sync.dma_start(out=FM, in_=fm_d.rearrange("b c y x -> (b c) (y x)"))

        mx = stats.tile([NP, 1], F32)
        nc.vector.reduce_max(mx, FM, axis=mybir.AxisListType.X)
        nmx = stats.tile([NP, 1], F32)
        nc.vector.tensor_scalar_mul(nmx, mx, -1.0)
        Z = stats.tile([NP, 1], F32)
        nc.scalar.activation(FM, FM, mybir.ActivationFunctionType.Exp,
                             bias=nmx, scale=1.0, accum_out=Z)
        Zr = stats.tile([NP, 1], F32)
        nc.vector.reciprocal(Zr, Z)
        # full normalization (keeps boundary decisions aligned w/ reference)
        nc.vector.tensor_scalar(FM, FM, Zr, None, op0=ALU.mult)

        SMyx = FM.rearrange("p (y x) -> p y x", y=YF)
        rowsum = stats.tile([NP, YF], F32)
        nc.vector.reduce_sum(rowsum, SMyx, axis=mybir.AxisListType.X)
        colsum = stats.tile([NP, XF], F32)
        nc.vector.reduce_sum(colsum, SMyx.rearrange("p y x -> p x y"),
                             axis=mybir.AxisListType.X)

        mxw = stats.tile([NP, XF], F32)
        nc.vector.tensor_tensor(mxw, colsum, gxc, op=ALU.mult)
        mean_x = stats.tile([NP, 1], F32)
        nc.vector.reduce_sum(mean_x, mxw, axis=mybir.AxisListType.X)
        myw = stats.tile([NP, YF], F32)
        nc.vector.tensor_tensor(myw, rowsum, gyc, op=ALU.mult)
        mean_y = stats.tile([NP, 1], F32)
        nc.vector.reduce_sum(mean_y, myw, axis=mybir.AxisListType.X)

        dxt = stats.tile([NP, XF], F32)
        nc.vector.tensor_scalar(dxt, gxc, mean_x, None, op0=ALU.subtract)
        dxw = stats.tile([NP, XF], F32)
        nc.vector.tensor_tensor(dxw, dxt, colsum, op=ALU.mult)
        sx = stats.tile([NP, 1], F32)
        nc.vector.reduce_sum(sx, dxw, axis=mybir.AxisListType.X,
                             apply_absolute_value=True)
        dyt = stats.tile([NP, YF], F32)
        nc.vector.tensor_scalar(dyt, gyc, mean_y, None, op0=ALU.subtract)
        dyw = stats.tile([NP, YF], F32)
        nc.vector.tensor_tensor(dyw, dyt, rowsum, op=ALU.mult)
        sy = stats.tile([NP, 1], F32)
        nc.vector.reduce_sum(sy, dyw, axis=mybir.AxisListType.X,
                             apply_absolute_value=True)
        scale_t = stats.tile([NP, 1], F32)
        nc.vector.tensor_tensor(scale_t, sx, sy, op=ALU.add)

        # stat outputs
        nc.sync.dma_start(out=meanx_d.rearrange("b c -> (b c)").unsqueeze(-1), in_=mean_x)
        nc.sync.dma_start(out=meany_d.rearrange("b c -> (b c)").unsqueeze(-1), in_=mean_y)
        nc.sync.dma_start(out=scale_d.rearrange("b c -> (b c)").unsqueeze(-1), in_=scale_t)

        # ---- sampling coords (pixel space), masked ----
        def coords(mean_t, name):
            ind = stats.tile([NP, OY], F32, name=f"ind_{name}")
            # ind = grid*scale + mean   (same op order as reference)
            nc.vector.tensor_scalar(ind, gridc, scale_t, mean_t,
                                    op0=ALU.mult, op1=ALU.add)
            r = stats.tile([NP, OY], F32, name=f"r_{name}")
            # r = (ind + 1) * 112
            nc.vector.tensor_scalar(r, ind, 1.0, YI / 2.0,
                                    op0=ALU.add, op1=ALU.mult)
            m1 = stats.tile([NP, OY], F32, name=f"m1_{name}")
            nc.vector.tensor_scalar(m1, r, 0.0, None, op0=ALU.is_ge)
            m2 = stats.tile([NP, OY], F32, name=f"m2_{name}")
            nc.vector.tensor_scalar(m2, r, float(YI - 1), None, op0=ALU.is_le)
            # exact-integer coords produce 0 in the reference (ceil==floor):
            # round(r) via the 2^23 trick, then ne = 1 - (r == round(r))
            rnd = stats.tile([NP, OY], F32, name=f"rnd_{name}")
            nc.vector.tensor_scalar(rnd, r, 8388608.0, -8388608.0,
                                    op0=ALU.add, op1=ALU.add)
            eq = stats.tile([NP, OY], F32, name=f"eq_{name}")
            nc.vector.tensor_tensor(eq, r, rnd, op=ALU.is_equal)
            ne = stats.tile([NP, OY], F32, name=f"ne_{name}")
            nc.vector.tensor_scalar(ne, eq, -1.0, 1.0, op0=ALU.mult, op1=ALU.add)
            m = stats.tile([NP, OY], F32, name=f"m_{name}")
            nc.vector.tensor_tensor(m, m1, m2, op=ALU.mult)
            nc.vector.tensor_tensor(m, m, ne, op=ALU.mult)
            big = stats.tile([NP, OY], F32, name=f"big_{name}")
            nc.vector.tensor_scalar(big, m, -OOB, OOB, op0=ALU.mult, op1=ALU.add)
            rm = stats.tile([NP, OY], F32, name=f"rm_{name}")
            nc.vector.tensor_tensor(rm, r, m, op=ALU.mult)
            nc.vector.tensor_tensor(rm, rm, big, op=ALU.add)
            return rm

        ym = coords(mean_y, "y")
        xm = coords(mean_x, "x")

        KS = [(K0, 0, nio0), (K1, K0, nio1)]

        # ---- per-batch: image load, W build, stage 1, stage 2 ----
        for b in range(BL):
            # flatten masked coords to a single partition-0 row (1, 2048)
            # (partition_broadcast requires base partition 0)
            fy = flats.tile([1, NCI], F32, tag="fy")
            nc.sync.dma_start(
                out=fy.rearrange("p (c i) -> p c i", c=C),
                in_=ym[b * C:(b + 1) * C, :])
            fx = flats.tile([1, NCI], F32, tag="fx")
            nc.sync.dma_start(
                out=fx.rearrange("p (c i) -> p c i", c=C),
                in_=xm[b * C:(b + 1) * C, :])
            # image slab for this batch (src rows split 128 + 96)
            img0 = imgp.tile([K0, CI, XI], MM_DT, tag="img0")
            img1 = imgp.tile([K1, CI, XI], MM_DT, tag="img1")
            ieng = nc.sync if MM_DT == F32 else nc.gpsimd
            for ci in range(CI):
                ieng.dma_start(out=img0[:, ci, :], in_=images_d[b, ci, 0:K0, :])
                ieng.dma_start(out=img1[:, ci, :], in_=images_d[b, ci, K0:YI, :])

            # broadcast coord rows to all partitions
            yrep = reps.tile([K0, NCI], F32, tag="yrep")
            nc.gpsimd.partition_broadcast(yrep, fy)
            xrep = reps.tile([K0, NCI], F32, tag="xrep")
            nc.gpsimd.partition_broadcast(xrep, fx)

            wy = []
            wx = []
            for k, (kp, koff, nio) in enumerate(KS):
                a = wabs.tile([K0, NCI], F32, tag="wabs")
                # |r - s| in one ACT pass: Abs(yrep + (-iota))
                nc.scalar.activation(a[0:kp, :], yrep[0:kp, :],
                                     mybir.ActivationFunctionType.Abs,
                                     bias=nio, scale=1.0)
                w = wp.tile([kp, NCI], MM_DT, tag=f"wy{k}")
                # -hat = min(|t| - 1, 0)
                nc.vector.tensor_scalar(w, a[0:kp, :], 1.0, 0.0,
                                        op0=ALU.subtract, op1=ALU.min)
                wy.append(w)

                a2 = wabs.tile([K0, NCI], F32, tag="wabs")
                nc.scalar.activation(a2[0:kp, :], xrep[0:kp, :],
                                     mybir.ActivationFunctionType.Abs,
                                     bias=nio, scale=1.0)
                w2 = wp.tile([kp, NCI], MM_DT, tag=f"wx{k}")
                nc.vector.tensor_scalar(w2, a2[0:kp, :], 1.0, 0.0,
                                        op0=ALU.subtract, op1=ALU.min)
                wx.append(w2)

            # stage 1: o1[x, (c,i)] = sum_y img[y, x] * WyT[y, (c,i)]
            o1 = []
            for ci in range(CI):
                o1a = o1p.tile([K0, NCI], MM_DT, tag=f"o1a{ci}")
                o1b = o1p.tile([K1, NCI], MM_DT, tag=f"o1b{ci}")
                for m, (mp, moff, osb) in enumerate([(K0, 0, o1a), (K1, K0, o1b)]):
                    for h in range(2):          # halves of 2048
                        pA = ps1.tile([128, 1024], F32, tag="pA")
                        for q in range(2):      # 512-wide matmuls
                            nlo = h * 1024 + q * 512
                            for k, (kp, koff, io) in enumerate(KS):
                                nc.tensor.matmul(
                                    pA[0:mp, q * 512:(q + 1) * 512],
                                    (img0 if k == 0 else img1)[:, ci, moff:moff + mp],
                                    wy[k][:, nlo:nlo + 512],
                                    start=(k == 0), stop=(k == 1),
                                )
                        # drain (cast to MM_DT)
                        dst = osb[:, h * 1024:(h + 1) * 1024]
                        if (m + h) % 2 == 0:
                            nc.scalar.copy(dst, pA[0:mp, :])
                        else:
                            nc.vector.tensor_copy(dst, pA[0:mp, :])
                o1.append((o1a, o1b))

            # stage 2: roi[b,ci,c,i,j] = sum_x o1[x,(c,i)] * WxT[x,(c,j)]
            for c in range(C):
                pB = ps2.tile([128, CI, OX], F32, tag="pB")
                for ci in range(CI):
                    o1a, o1b = o1[ci]
                    for k in range(2):
                        lhs = (o1a if k == 0 else o1b)[:, c * OY:(c + 1) * OY]
                        rhs = wx[k][:, c * OX:(c + 1) * OX]
                        nc.tensor.matmul(
                            pB[:, ci, :], lhs, rhs,
                            start=(k == 0), stop=(k == 1),
                        )
                rc = rcp.tile([128, CI, OX], F32, tag="rc")
                if c % 2 == 0:
                    nc.scalar.copy(rc, pB)
                else:
                    nc.vector.tensor_copy(rc, pB)
                nc.sync.dma_start(
                    out=roi_d[b, :, c, :, :].rearrange("ci i j -> i ci j"),
                    in_=rc)

    nc.compile()
    return nc


def _get_nc():
    if "nc" not in _CACHE:
        _CACHE["nc"] = _build_program()
    return _CACHE["nc"]


def _consts_np():
    return {
        "c_gx": np.linspace(-1, 1, XF, dtype=np.float64).astype(np.float32),
        "c_gy": np.linspace(-1, 1, YF, dtype=np.float64).astype(np.float32),
        "c_grid": np.linspace(-1, 1, OY, dtype=np.float64).astype(np.float32),
        "c_niota": -np.arange(YI, dtype=np.float32),
    }


def kernel(images, feature_map):
    global LAST_EXEC_NS, LAST_RESULTS
    images = np.ascontiguousarray(np.asarray(images, dtype=np.float32))
    feature_map = np.ascontiguousarray(np.asarray(feature_map, dtype=np.float32))
    assert images.shape == (B, CI, YI, XI)
    assert feature_map.shape == (B, C, YF, XF)

    nc = _get_nc()
    consts = _consts_np()
    in_maps = []
    for s in range(NCORES):
        lo, hi = s * BL, (s + 1) * BL
        m = {"images": np.ascontiguousarray(images[lo:hi]),
             "feature_map": np.ascontiguousarray(feature_map[lo:hi])}
        m.update(consts)
        in_maps.append(m)

    trace = bool(int(os.environ.get("KERNEL_TRACE", "0")))
    res = run_bass_kernel_spmd(nc, in_maps, core_ids=list(range(NCORES)),
                               trace=trace)
    LAST_EXEC_NS = res.exec_time_ns
    LAST_RESULTS = res

    roi = np.concatenate([res.results[i]["roi"] for i in range(NCORES)], axis=0)
    mean_x = np.concatenate([res.results[i]["mean_x"] for i in range(NCORES)], axis=0)
    mean_y = np.concatenate([res.results[i]["mean_y"] for i in range(NCORES)], axis=0)
    scale = np.concatenate([res.results[i]["scale"] for i in range(NCORES)], axis=0)
    return roi, mean_x, mean_y, scale


# revision 13
# speedup vs baseline: 2.0983x; 1.1740x over previous
"""Trainium2 Bass kernel for soft-argmax ROI bilinear sampling.

Problem (hardcoded):
  images:      (64, 3, 224, 224) f32
  feature_map: (64, 16, 56, 56) f32
  outputs: roi (64, 3, 16, 128, 128), mean_x/mean_y/scale (64, 16), all f32

Math: per (b, c): softmax over the 56x56 feature map gives soft-argmax
mean_x/mean_y and an L1 spread 'scale'.  A 128x128 sampling grid is
bilinear-sampled from images[b].  The sample coordinates are separable
(x coords depend only on output col j, y coords only on output row i), so
bilinear sampling == two small dense matmuls with "hat" weight matrices:
    roi[b,ci,c] = Wy[b,c] @ images[b,ci] @ Wx[b,c]^T
where W[o, s] = relu(1 - |r_o - s|) for in-range coords, 0 otherwise
(matching the reference's clipped ceil/floor behavior, including coords
exactly on integers producing 0).  We build W negated (min(|t|-1, 0)) so a
single fused tensor_scalar produces it; the two negations cancel.

Sharding: pure data parallel over batch, 8 batches per core x 8 cores.
"""

import os
from contextlib import ExitStack

import numpy as np

import concourse.bass as bass
import concourse.bacc as bacc
import concourse.tile as tile
import concourse.mybir as mybir
from concourse.bass_utils import run_bass_kernel_spmd

F32 = mybir.dt.float32
ALU = mybir.AluOpType

# ---- problem dims (hardcoded per contract) ----
B, CI, YI, XI = 64, 3, 224, 224
C, YF, XF = 16, 56, 56
OY = OX = 128
NCORES = 8
BL = B // NCORES          # 8 local batches
NP = BL * C               # 128 = (b,c) pairs per core -> partition dim
FDIM = YF * XF            # 3136

# K tiling of the 224-long source axes: 128 + 96
K0, K1 = 128, 96
NCI = 2048                # (c, i) packed free size = C * OY

# matmul dtype for the heavy stages (images / W / intermediate).
# float32 = exact; bfloat16 = ~2x faster PE, ~0.5% rel err.
MM_DT = mybir.dt.bfloat16
OOB = 1.0e4

_CACHE = {}

LAST_EXEC_NS = None
LAST_RESULTS = None


def _build_program():
    nc = bacc.Bacc("TRN2", target_bir_lowering=False, debug=False,
                   enable_asserts=False, num_devices=NCORES)

    # ---- DRAM I/O ----
    images_d = nc.dram_tensor("images", [BL, CI, YI, XI], F32, kind="ExternalInput").ap()
    fm_d = nc.dram_tensor("feature_map", [BL, C, YF, XF], F32, kind="ExternalInput").ap()
    c_gx = nc.dram_tensor("c_gx", [XF], F32, kind="ExternalInput")
    c_gy = nc.dram_tensor("c_gy", [YF], F32, kind="ExternalInput")
    c_grid = nc.dram_tensor("c_grid", [OY], F32, kind="ExternalInput")
    c_niota = nc.dram_tensor("c_niota", [YI], F32, kind="ExternalInput")

    roi_d = nc.dram_tensor("roi", [BL, CI, C, OY, OX], F32, kind="ExternalOutput").ap()
    meanx_d = nc.dram_tensor("mean_x", [BL, C], F32, kind="ExternalOutput").ap()
    meany_d = nc.dram_tensor("mean_y", [BL, C], F32, kind="ExternalOutput").ap()
    scale_d = nc.dram_tensor("scale", [BL, C], F32, kind="ExternalOutput").ap()

    def bc(handle, n, p=128):
        # DRAM vector broadcast across p partitions: AP [[0,p],[1,n]]
        return bass.AP(tensor=handle, offset=0, ap=[[0, p], [1, n]])

    with tile.TileContext(nc) as tc, ExitStack() as ctx:
        consts = ctx.enter_context(tc.tile_pool(name="consts", bufs=1))
        fmp = ctx.enter_context(tc.tile_pool(name="fmp", bufs=1))
        stats = ctx.enter_context(tc.tile_pool(name="stats", bufs=1))
        imgp = ctx.enter_context(tc.tile_pool(name="imgp", bufs=2))
        flats = ctx.enter_context(tc.tile_pool(name="flats", bufs=2))
        reps = ctx.enter_context(tc.tile_pool(name="reps", bufs=2))
        wabs = ctx.enter_context(tc.tile_pool(name="wabs", bufs=2))
        wp = ctx.enter_context(tc.tile_pool(name="wp", bufs=2))
        o1p = ctx.enter_context(tc.tile_pool(name="o1p", bufs=2))
        rcp = ctx.enter_context(tc.tile_pool(name="rcp", bufs=4))
        ps1 = ctx.enter_context(tc.tile_pool(name="ps1", bufs=2, space="PSUM"))
        ps2 = ctx.enter_context(tc.tile_pool(name="ps2", bufs=2, space="PSUM"))

        # ---- constants to SBUF ----
        gxc = consts.tile([NP, XF], F32)
        nc.sync.dma_start(out=gxc, in_=bc(c_gx, XF))
        gyc = consts.tile([NP, YF], F32)
        nc.sync.dma_start(out=gyc, in_=bc(c_gy, YF))
        gridc = consts.tile([NP, OY], F32)
        nc.sync.dma_start(out=gridc, in_=bc(c_grid, OY))
        nio0 = consts.tile([K0, 1], F32)
        nc.sync.dma_start(out=nio0, in_=c_niota.ap()[0:K0].unsqueeze(-1))
        nio1 = consts.tile([K1, 1], F32)
        nc.sync.dma_start(out=nio1, in_=c_niota.ap()[K0:YI].unsqueeze(-1))

        # ---- feature map as (128, 3136); softmax done in-place ----
        FM = fmp.tile([NP, FDIM], F32)
        nc.sync.dma_start(out=FM, in_=fm_d.rearrange("b c y x -> (b c) (y x)"))

        mx = stats.tile([NP, 1], F32)
        nc.vector.reduce_max(mx, FM, axis=mybir.AxisListType.X)
        nmx = stats.tile([NP, 1], F32)
        nc.vector.tensor_scalar_mul(nmx, mx, -1.0)
        Z = stats.tile([NP, 1], F32)
        nc.scalar.activation(FM, FM, mybir.ActivationFunctionType.Exp,
                             bias=nmx, scale=1.0, accum_out=Z)
        Zr = stats.tile([NP, 1], F32)
        nc.vector.reciprocal(Zr, Z)
        # full normalization (keeps boundary decisions aligned w/ reference)
        nc.vector.tensor_scalar(FM, FM, Zr, None, op0=ALU.mult)

        SMyx = FM.rearrange("p (y x) -> p y x", y=YF)
        rowsum = stats.tile([NP, YF], F32)
        nc.vector.reduce_sum(rowsum, SMyx, axis=mybir.AxisListType.X)
        colsum = stats.tile([NP, XF], F32)
        nc.vector.reduce_sum(colsum, SMyx.rearrange("p y x -> p x y"),
                             axis=mybir.AxisListType.X)

        mxw = stats.tile([NP, XF], F32)
        nc.vector.tensor_tensor(mxw, colsum, gxc, op=ALU.mult)
        mean_x = stats.tile([NP, 1], F32)
        nc.vector.reduce_sum(mean_x, mxw, axis=mybir.AxisListType.X)
        myw = stats.tile([NP, YF], F32)
        nc.vector.tensor_tensor(myw, rowsum, gyc, op=ALU.mult)
        mean_y = stats.tile([NP, 1], F32)
        nc.vector.reduce_sum(mean_y, myw, axis=mybir.AxisListType.X)

        dxt = stats.tile([NP, XF], F32)
        nc.vector.tensor_scalar(dxt, gxc, mean_x, None, op0=ALU.subtract)
        dxw = stats.tile([NP, XF], F32)
        nc.vector.tensor_tensor(dxw, dxt, colsum, op=ALU.mult)
        sx = stats.tile([NP, 1], F32)
        nc.vector.reduce_sum(sx, dxw, axis=mybir.AxisListType.X,
                             apply_absolute_value=True)
        dyt = stats.tile([NP, YF], F32)
        nc.vector.tensor_scalar(dyt, gyc, mean_y, None, op0=ALU.subtract)
        dyw = stats.tile([NP, YF], F32)
        nc.vector.tensor_tensor(dyw, dyt, rowsum, op=ALU.mult)
        sy = stats.tile([NP, 1], F32)
        nc.vector.reduce_sum(sy, dyw, axis=mybir.AxisListType.X,
                             apply_absolute_value=True)
        scale_t = stats.tile([NP, 1], F32)
        nc.vector.tensor_tensor(scale_t, sx, sy, op=ALU.add)

        # stat outputs
        nc.sync.dma_start(out=meanx_d.rearrange("b c -> (b c)").unsqueeze(-1), in_=mean_x)
        nc.sync.dma_start(out=meany_d.rearrange("b c -> (b c)").unsqueeze(-1), in_=mean_y)
        nc.sync.dma_start(out=scale_d.rearrange("b c -> (b c)").unsqueeze(-1), in_=scale_t)

        # ---- sampling coords (pixel space), masked ----
        def coords(mean_t, name):
            ind = stats.tile([NP, OY], F32, name=f"ind_{name}")
            # ind = grid*scale + mean   (same op order as reference)
            nc.vector.tensor_scalar(ind, gridc, scale_t, mean_t,
                                    op0=ALU.mult, op1=ALU.add)
            r = stats.tile([NP, OY], F32, name=f"r_{name}")
            # r = (ind + 1) * 112
            nc.vector.tensor_scalar(r, ind, 1.0, YI / 2.0,
                                    op0=ALU.add, op1=ALU.mult)
            m1 = stats.tile([NP, OY], F32, name=f"m1_{name}")
            nc.vector.tensor_scalar(m1, r, 0.0, None, op0=ALU.is_ge)
            m2 = stats.tile([NP, OY], F32, name=f"m2_{name}")
            nc.vector.tensor_scalar(m2, r, float(YI - 1), None, op0=ALU.is_le)
            # exact-integer coords produce 0 in the reference (ceil==floor):
            # round(r) via the 2^23 trick, then ne = 1 - (r == round(r))
            rnd = stats.tile([NP, OY], F32, name=f"rnd_{name}")
            nc.vector.tensor_scalar(rnd, r, 8388608.0, -8388608.0,
                                    op0=ALU.add, op1=ALU.add)
            eq = stats.tile([NP, OY], F32, name=f"eq_{name}")
            nc.vector.tensor_tensor(eq, r, rnd, op=ALU.is_equal)
            ne = stats.tile([NP, OY], F32, name=f"ne_{name}")
            nc.vector.tensor_scalar(ne, eq, -1.0, 1.0, op0=ALU.mult, op1=ALU.add)
            m = stats.tile([NP, OY], F32, name=f"m_{name}")
            nc.vector.tensor_tensor(m, m1, m2, op=ALU.mult)
            nc.vector.tensor_tensor(m, m, ne, op=ALU.mult)
            big = stats.tile([NP, OY], F32, name=f"big_{name}")
            nc.vector.tensor_scalar(big, m, -OOB, OOB, op0=ALU.mult, op1=ALU.add)
            rm = stats.tile([NP, OY], F32, name=f"rm_{name}")
            nc.vector.tensor_tensor(rm, r, m, op=ALU.mult)
            nc.vector.tensor_tensor(rm, rm, big, op=ALU.add)
            return rm

        ym = coords(mean_y, "y")
        xm = coords(mean_x, "x")

        KS = [(K0, 0, nio0), (K1, K0, nio1)]

        # ---- per-batch: image load, W build, stage 1, stage 2 ----
        for b in range(BL):
            # flatten masked coords to a single partition-0 row (1, 2048)
            # (partition_broadcast requires base partition 0)
            fy = flats.tile([1, NCI], F32, tag="fy")
            nc.sync.dma_start(
                out=fy.rearrange("p (c i) -> p c i", c=C),
                in_=ym[b * C:(b + 1) * C, :])
            fx = flats.tile([1, NCI], F32, tag="fx")
            nc.sync.dma_start(
                out=fx.rearrange("p (c i) -> p c i", c=C),
                in_=xm[b * C:(b + 1) * C, :])
            # image slab for this batch (src rows split 128 + 96)
            img0 = imgp.tile([K0, CI, XI], MM_DT, tag="img0")
            img1 = imgp.tile([K1, CI, XI], MM_DT, tag="img1")
            ieng = nc.sync if MM_DT == F32 else nc.gpsimd
            for ci in range(CI):
                ieng.dma_start(out=img0[:, ci, :], in_=images_d[b, ci, 0:K0, :])
                ieng.dma_start(out=img1[:, ci, :], in_=images_d[b, ci, K0:YI, :])

            # broadcast coord rows to all partitions
            yrep = reps.tile([K0, NCI], F32, tag="yrep")
            nc.gpsimd.partition_broadcast(yrep, fy)
            xrep = reps.tile([K0, NCI], F32, tag="xrep")
            nc.gpsimd.partition_broadcast(xrep, fx)

            wy = []
            wx = []
            for k, (kp, koff, nio) in enumerate(KS):
                a = wabs.tile([K0, NCI], F32, tag="wabs")
                # |r - s| in one ACT pass: Abs(yrep + (-iota))
                nc.scalar.activation(a[0:kp, :], yrep[0:kp, :],
                                     mybir.ActivationFunctionType.Abs,
                                     bias=nio, scale=1.0)
                w = wp.tile([kp, NCI], MM_DT, tag=f"wy{k}")
                # -hat = min(|t| - 1, 0)
                nc.vector.tensor_scalar(w, a[0:kp, :], 1.0, 0.0,
                                        op0=ALU.subtract, op1=ALU.min)
                wy.append(w)

                a2 = wabs.tile([K0, NCI], F32, tag="wabs")
                nc.scalar.activation(a2[0:kp, :], xrep[0:kp, :],
                                     mybir.ActivationFunctionType.Abs,
                                     bias=nio, scale=1.0)
                w2 = wp.tile([kp, NCI], MM_DT, tag=f"wx{k}")
                nc.vector.tensor_scalar(w2, a2[0:kp, :], 1.0, 0.0,
                                        op0=ALU.subtract, op1=ALU.min)
                wx.append(w2)

            # stage 1: o1[x, (c,i)] = sum_y img[y, x] * WyT[y, (c,i)]
            o1 = []
            for ci in range(CI):
                o1a = o1p.tile([K0, NCI], MM_DT, tag=f"o1a{ci}")
                o1b = o1p.tile([K1, NCI], MM_DT, tag=f"o1b{ci}")
                for m, (mp, moff, osb) in enumerate([(K0, 0, o1a), (K1, K0, o1b)]):
                    for h in range(2):          # halves of 2048
                        pA = ps1.tile([128, 1024], F32, tag="pA")
                        for q in range(2):      # 512-wide matmuls
                            nlo = h * 1024 + q * 512
                            for k, (kp, koff, io) in enumerate(KS):
                                nc.tensor.matmul(
                                    pA[0:mp, q * 512:(q + 1) * 512],
                                    (img0 if k == 0 else img1)[:, ci, moff:moff + mp],
                                    wy[k][:, nlo:nlo + 512],
                                    start=(k == 0), stop=(k == 1),
                                )
                        # drain (cast to MM_DT)
                        dst = osb[:, h * 1024:(h + 1) * 1024]
                        if (m + h) % 2 == 0:
                            nc.scalar.copy(dst, pA[0:mp, :])
                        else:
                            nc.vector.tensor_copy(dst, pA[0:mp, :])
                o1.append((o1a, o1b))

            # stage 2: roi[b,ci,c,i,j] = sum_x o1[x,(c,i)] * WxT[x,(c,j)]
            # process c in pairs: one 2-bank PSUM tile, one drain, one DMA
            for cp in range(C // 2):
                pB = ps2.tile([128, 2, CI, OX], F32, tag="pB")
                for dc in range(2):
                    c = 2 * cp + dc
                    for ci in range(CI):
                        o1a, o1b = o1[ci]
                        for k in range(2):
                            lhs = (o1a if k == 0 else o1b)[:, c * OY:(c + 1) * OY]
                            rhs = wx[k][:, c * OX:(c + 1) * OX]
                            nc.tensor.matmul(
                                pB[:, dc, ci, :], lhs, rhs,
                                start=(k == 0), stop=(k == 1),
                            )
                rc = rcp.tile([128, 2, CI, OX], F32, tag="rc")
                if cp % 2 == 0:
                    nc.scalar.copy(rc, pB)
                else:
                    nc.vector.tensor_copy(rc, pB)
                for dc in range(2):
                    nc.sync.dma_start(
                        out=roi_d[b, :, 2 * cp + dc, :, :].rearrange(
                            "ci i j -> i ci j"),
                        in_=rc[:, dc, :, :])

    nc.compile()
    return nc


def _get_nc():
    if "nc" not in _CACHE:
        _CACHE["nc"] = _build_program()
    return _CACHE["nc"]


def _consts_np():
    return {
        "c_gx": np.linspace(-1, 1, XF, dtype=np.float64).astype(np.float32),
        "c_gy": np.linspace(-1, 1, YF, dtype=np.float64).astype(np.float32),
        "c_grid": np.linspace(-1, 1, OY, dtype=np.float64).astype(np.float32),
        "c_niota": -np.arange(YI, dtype=np.float32),
    }


def kernel(images, feature_map):
    global LAST_EXEC_NS, LAST_RESULTS
    images = np.ascontiguousarray(np.asarray(images, dtype=np.float32))
    feature_map = np.ascontiguousarray(np.asarray(feature_map, dtype=np.float32))
    assert images.shape == (B, CI, YI, XI)
    assert feature_map.shape == (B, C, YF, XF)

    nc = _get_nc()
    consts = _consts_np()
    in_maps = []
    for s in range(NCORES):
        lo, hi = s * BL, (s + 1) * BL
        m = {"images": np.ascontiguousarray(images[lo:hi]),
             "feature_map": np.ascontiguousarray(feature_map[lo:hi])}
        m.update(consts)
        in_maps.append(m)

    trace = bool(int(os.environ.get("KERNEL_TRACE", "0")))
    res = run_bass_kernel_spmd(nc, in_maps, core_ids=list(range(NCORES)),
                               trace=trace)
    LAST_EXEC_NS = res.exec_time_ns
    LAST_RESULTS = res

    roi = np.concatenate([res.results[i]["roi"] for i in range(NCORES)], axis=0)
    mean_x = np.concatenate([res.results[i]["mean_x"] for i in range(NCORES)], axis=0)
    mean_y = np.concatenate([res.results[i]["mean_y"] for i in range(NCORES)], axis=0)
    scale = np.concatenate([res.results[i]["scale"] for i in range(NCORES)], axis=0)
    return roi, mean_x, mean_y, scale
